# revision 1
# baseline (speedup 1.0000x reference)
"""Trainium2 Bass kernel for the BAN (bilinear attention network) problem.

Math (per batch b, eval mode):
    hq = emb[he_ques] @ Wq + bq                  [NQ, H]
    hk = emb[he_kg]   @ Wk + bk                  [NK, H]
    logits[g,q,k] = sum_d hq[q,d] Watt[d,g] hk[k,d]   (+ batt[g], which cancels
                                                       in the joint softmax)
    att = softmax over flattened (q,k) per (b,g)
    pooled[g,d] = sum_{q,k} hq[q,d] att[g,q,k] hk[k,d]
    out = pooled.flat @ Wout + bout;  sim = out @ glove.T;  log_softmax(sim)

Distribution: pure data parallel over batch, 8 samples per core on 8 cores.
All weights replicated. No collectives.

Matmul operands use float32r (single-pass PE mode, ~1.3e-4 matmul accuracy
vs 4x slower two-pass fp32); accumulation stays fp32 in PSUM.

Layout strategy (per core, B_loc=8):
  - emb is host-augmented with a ones column (E=301) so projection biases
    ride in the matmul; tokens gathered on-device via indirect DMA:
    X [token, 301], PE-transposed to X.T [E, token].
  - hq.T [d, tok] and hk.T [d, tok] from lhsT=W, rhs=X.T
  - hk   [tok, d] from lhsT=X.T, rhs=W
  - logits.T [k, (g,q)] = (hk.T tiles).T @ (hq.T * Watt[:,g])  -> logits are
    O(+-6), so exp() without max subtraction is safe in fp32; the joint
    softmax normalization Z_g = sum E is applied to pooled.T per sample.
  - u.T [d, (g,q)] = (hk tiles).T @ E.T;  v = u.T * hq.T (bcast over g);
    pooled.T[d, g] = reduce_q v, scaled by 1/Z_g.
  - out [8, 300] = (pooled.T as lhsT).T @ Wout tiles;  sim [8, 4000] via
    lhsT=out.T (PE transpose), rhs=glove.T;  log-softmax on [8, 4000].
  - Wout/glove.T tile DMAs are emitted up-front so they stream into SBUF
    while the attention loop runs.
"""

import sys

if "/opt/trn_rl_repo" not in sys.path:
    sys.path.insert(0, "/opt/trn_rl_repo")

import numpy as np

import concourse.bass as bass
import concourse.mybir as mybir
import concourse.tile as tile
from concourse import bacc
from concourse.bass_utils import run_bass_kernel_spmd

F32 = mybir.dt.float32
F32R = mybir.dt.float32r
I32 = mybir.dt.int32
AX = mybir.AxisListType
OP = mybir.AluOpType
AF = mybir.ActivationFunctionType

N_CORES = 8
VOCAB = 20000
E = 300          # word embedding size
EA = E + 1       # augmented with ones column (bias trick)
H = 1024         # hidden
G = 8            # heads
N_OUT = 300
N_ANS = 4000
B, NQ, NK = 64, 32, 256
BL = B // N_CORES            # 8 samples per core
TQ = BL * NQ                 # 256 q tokens per core
TK = BL * NK                 # 2048 k tokens per core
TQ_TILES = TQ // 128         # 2
TK_TILES = TK // 128         # 16
DT = H // 128                # 8 d-tiles
E_CHUNKS = (128, 128, EA - 256)   # (128, 128, 45)
N_CHUNKS = (128, 128, N_OUT - 256)
NA_CH = 8                    # sim computed in 8 chunks of 500
NA_W = N_ANS // NA_CH        # 500
NWOUT = G * DT               # 64 Wout k-tiles


def build_kernel():
    nc = bacc.Bacc("TRN2", target_bir_lowering=False, debug=False,
                   num_devices=N_CORES)

    # ---- DRAM I/O ----
    emb_d = nc.dram_tensor("emb", [VOCAB, EA], F32R, kind="ExternalInput").ap()
    idxq_d = nc.dram_tensor("idx_q", [128, TQ_TILES], I32, kind="ExternalInput").ap()
    idxk_d = nc.dram_tensor("idx_k", [128, TK_TILES], I32, kind="ExternalInput").ap()
    wq_d = nc.dram_tensor("wq", [EA, H], F32R, kind="ExternalInput").ap()
    wk_d = nc.dram_tensor("wk", [EA, H], F32R, kind="ExternalInput").ap()
    watt_d = nc.dram_tensor("watt", [128, DT, G], F32, kind="ExternalInput").ap()
    wout_d = nc.dram_tensor("wout", [G * H, N_OUT], F32R, kind="ExternalInput").ap()
    bout_d = nc.dram_tensor("bout", [BL, N_OUT], F32, kind="ExternalInput").ap()
    glovet_d = nc.dram_tensor("glovet", [N_OUT, N_ANS], F32R,
                              kind="ExternalInput").ap()
    ident_d = nc.dram_tensor("ident", [128, 128], F32R, kind="ExternalInput").ap()
    ones_d = nc.dram_tensor("ones_col", [128, 1], F32R, kind="ExternalInput").ap()
    out_d = nc.dram_tensor("out", [BL, N_ANS], F32, kind="ExternalOutput").ap()
    warm_d = nc.dram_tensor("warm", [1, 128], F32, kind="ExternalOutput").ap()

    with tile.TileContext(nc) as tc:
        import contextlib

        with contextlib.ExitStack() as ctx:
            consts = ctx.enter_context(tc.tile_pool(name="consts", bufs=1))
            wout_p = ctx.enter_context(tc.tile_pool(name="wout", bufs=26))
            glove_p = ctx.enter_context(tc.tile_pool(name="glove", bufs=2))
            actx = contextlib.ExitStack()
            xrow_p = actx.enter_context(tc.tile_pool(name="xrow", bufs=4))
            xkt_p = actx.enter_context(tc.tile_pool(name="xkt", bufs=2))
            hkt_p = actx.enter_context(tc.tile_pool(name="hkt", bufs=2))
            hk_p = actx.enter_context(tc.tile_pool(name="hk", bufs=2))
            hqw_p = actx.enter_context(tc.tile_pool(name="hqw", bufs=2))
            et_p = actx.enter_context(tc.tile_pool(name="et", bufs=2))
            v_p = actx.enter_context(tc.tile_pool(name="v", bufs=2))
            zz_p = actx.enter_context(tc.tile_pool(name="zz", bufs=2))
            zn_p = actx.enter_context(tc.tile_pool(name="zn", bufs=2))
            mm_p = ctx.enter_context(tc.tile_pool(name="mm", bufs=4, space="PSUM"))
            lg_p = ctx.enter_context(tc.tile_pool(name="lg", bufs=2, space="PSUM"))
            up_p = ctx.enter_context(tc.tile_pool(name="up", bufs=2, space="PSUM"))

            # ---- constants into SBUF ----
            ident = consts.tile([128, 128], F32R, tag="ident")
            nc.sync.dma_start(ident[:], ident_d)
            # PE warm-up: ~5us of back-to-back matmuls on the identity while
            # the initial DMAs stream in, so HAM reaches K=8/8 before real work
            wps = mm_p.tile([128, 512], F32, tag="mm")
            for i in range(48):
                nc.tensor.matmul(wps[:, :128], lhsT=ident[:], rhs=ident[:],
                                 start=True, stop=True)
            warm_sb = consts.tile([1, 128], F32, tag="warm")
            nc.vector.tensor_copy(warm_sb[:], wps[:1, :128])
            nc.sync.dma_start(warm_d, warm_sb[:])

            idxq_sb = consts.tile([128, TQ_TILES], I32, tag="idxq")
            nc.sync.dma_start(idxq_sb[:], idxq_d)
            idxk_sb = consts.tile([128, TK_TILES], I32, tag="idxk")
            nc.sync.dma_start(idxk_sb[:], idxk_d)
            wq_sb = consts.tile([128, 3, H], F32R, tag="wq")
            wk_sb = consts.tile([128, 3, H], F32R, tag="wk")
            for c, rows in enumerate(E_CHUNKS):
                nc.sync.dma_start(wq_sb[:rows, c, :], wq_d[c * 128 : c * 128 + rows])
                nc.sync.dma_start(wk_sb[:rows, c, :], wk_d[c * 128 : c * 128 + rows])
            watt_sb = consts.tile([128, DT, G], F32, tag="watt")
            nc.sync.dma_start(watt_sb[:], watt_d)
            bout_sb = consts.tile([BL, N_OUT], F32, tag="bout")
            nc.sync.dma_start(bout_sb[:], bout_d)
            ones_sb = consts.tile([128, 1], F32R, tag="ones")
            nc.sync.dma_start(ones_sb[:], ones_d)

            # ---- early-issued weight streams for phases F/G ----
            wout_tiles = []
            for t in range(26):
                wtile = wout_p.tile([128, N_OUT], F32R, tag="wout")
                nc.sync.dma_start(wtile[:], wout_d[t * 128 : (t + 1) * 128, :])
                wout_tiles.append(wtile)
            glove_tiles = []
            for a in range(2):
                gt = glove_p.tile([128, 3, NA_W], F32R, tag="glove")
                for c, rows in enumerate(N_CHUNKS):
                    nc.sync.dma_start(
                        gt[:rows, c, :],
                        glovet_d[c * 128 : c * 128 + rows,
                                 a * NA_W : (a + 1) * NA_W],
                    )
                glove_tiles.append(gt)

            def gather_transpose(idx_sb, col, dst, dst_col):
                """Gather 128 emb rows (token tile) and write transpose into
                dst[:, c, dst_col*128:...] per E-chunk c. The emb ones column
                (E index 300) lands at partition 44 of chunk 2."""
                xrow = xrow_p.tile([128, EA], F32R, tag="xrow")
                nc.gpsimd.indirect_dma_start(
                    out=xrow[:],
                    out_offset=None,
                    in_=emb_d,
                    in_offset=bass.IndirectOffsetOnAxis(
                        ap=idx_sb[:, col : col + 1], axis=0
                    ),
                )
                for c, rows in enumerate(E_CHUNKS):
                    ps = mm_p.tile([128, 512], F32R, tag="mm")
                    nc.tensor.transpose(
                        ps[:rows, :128], xrow[:, c * 128 : c * 128 + rows], ident[:]
                    )
                    nc.any.tensor_copy(
                        out=dst[:rows, c, dst_col * 128 : (dst_col + 1) * 128],
                        in_=ps[:rows, :128],
                    )

            # ---- phase B: gather+transpose Xq -> xqT [128, 3, TQ] ----
            xqT = consts.tile([128, 3, TQ], F32R, tag="xqT")
            for t in range(TQ_TILES):
                gather_transpose(idxq_sb, t, xqT, t)

            # ---- phase C: hqT [128, DT, TQ] (fp32; only DVE consumes it) ----
            hqT = consts.tile([128, DT, TQ], F32, tag="hqT")
            for m in range(DT):
                ps = mm_p.tile([128, 512], F32, tag="mm")
                for c, rows in enumerate(E_CHUNKS):
                    nc.tensor.matmul(
                        ps[:, :TQ],
                        lhsT=wq_sb[:rows, c, m * 128 : (m + 1) * 128],
                        rhs=xqT[:rows, c, :],
                        start=(c == 0),
                        stop=(c == 2),
                    )
                nc.vector.tensor_copy(hqT[:, m, :], ps[:, :TQ])

            poT = consts.tile([128, DT, G, BL], F32R, tag="poT")

            # ---- phase D: attention, two samples per D2 batch ----
            for p in range(BL // 2):
                # D1: gather + transpose K tokens for samples 2p, 2p+1
                xkT = xkt_p.tile([128, 3, 512], F32R, tag="xkT")
                for t in range(4):
                    gather_transpose(idxk_sb, p * 4 + t, xkT, t)

                # D2: hkT for the pair [128, DT, 512]
                hkT = hkt_p.tile([128, DT, 512], F32R, tag="hkT")
                for m in range(DT):
                    ps = mm_p.tile([128, 512], F32, tag="mm")
                    for c, rows in enumerate(E_CHUNKS):
                        nc.tensor.matmul(
                            ps[:],
                            lhsT=wk_sb[:rows, c, m * 128 : (m + 1) * 128],
                            rhs=xkT[:rows, c, :],
                            start=(c == 0),
                            stop=(c == 2),
                        )
                    nc.any.tensor_copy(out=hkT[:, m, :], in_=ps[:])

                for bi in range(2):
                    b = p * 2 + bi

                    # D3: hk_b [128, 2, H] (token-partition layout)
                    hk = hk_p.tile([128, 2, H], F32R, tag="hk")
                    for t in range(2):
                        for nchunk in range(2):
                            ps = mm_p.tile([128, 512], F32, tag="mm")
                            for c, rows in enumerate(E_CHUNKS):
                                nc.tensor.matmul(
                                    ps[:],
                                    lhsT=xkT[
                                        :rows, c,
                                        (bi * 2 + t) * 128 : (bi * 2 + t + 1) * 128,
                                    ],
                                    rhs=wk_sb[
                                        :rows, c, nchunk * 512 : (nchunk + 1) * 512
                                    ],
                                    start=(c == 0),
                                    stop=(c == 2),
                                )
                            nc.any.tensor_copy(
                                out=hk[:, t, nchunk * 512 : (nchunk + 1) * 512],
                                in_=ps[:],
                            )

                    # D4: hqw [128, DT, G, NQ] = hqT(b) * watt (bcast over g)
                    hqw = hqw_p.tile([128, DT, G, NQ], F32R, tag="hqw")
                    nc.vector.tensor_tensor(
                        out=hqw[:],
                        in0=hqT[:, :, None, b * NQ : (b + 1) * NQ].to_broadcast(
                            [128, DT, G, NQ]
                        ),
                        in1=watt_sb[:, :, :, None].to_broadcast([128, DT, G, NQ]),
                        op=OP.mult,
                    )

                    # D5: logits.T [k, (g,q)] in PSUM: [128, 2, 256]
                    ps_l = lg_p.tile([128, 512], F32, tag="lg")
                    for kt in range(2):
                        for c in range(DT):
                            nc.tensor.matmul(
                                ps_l[:, kt * 256 : (kt + 1) * 256],
                                lhsT=hkT[
                                    :, c,
                                    bi * 256 + kt * 128 : bi * 256 + (kt + 1) * 128,
                                ],
                                rhs=hqw[:, c],
                                start=(c == 0),
                                stop=(c == DT - 1),
                            )

                    # D6: E = exp(logits), per-(g)-block row sums zz
                    et = et_p.tile([128, 2, G * NQ], F32R, tag="et")
                    zz = zz_p.tile([128, 2, G], F32R, tag="zz")
                    for kt in range(2):
                        nc.scalar.activation(
                            out=et[:, kt, :],
                            in_=ps_l[:, kt * 256 : (kt + 1) * 256],
                            func=AF.Exp,
                        )
                        with nc.allow_low_precision(reason="fp32r round of f32 sum"):
                            nc.vector.tensor_reduce(
                                out=zz[:, kt, :],
                                in_=et[:, kt].rearrange("p (g q) -> p g q", g=G),
                                axis=AX.X,
                                op=OP.add,
                            )

                    # D7: Z_g = sum over k-partitions; zinv_b = 1/Z broadcast
                    ps_z = mm_p.tile([128, 512], F32, tag="mm")
                    for kt in range(2):
                        nc.tensor.matmul(
                            ps_z[:1, :G],
                            lhsT=ones_sb[:],
                            rhs=zz[:, kt, :],
                            start=(kt == 0),
                            stop=(kt == 1),
                        )
                    zinv = zn_p.tile([1, G], F32, tag="zinv")
                    nc.vector.reciprocal(zinv[:1, :], ps_z[:1, :G])
                    zbro = zn_p.tile([128, G], F32, tag="zbro")
                    nc.gpsimd.partition_broadcast(zbro[:], zinv[:1, :], channels=128)

                    # D8: u.T, v, pooled partial sums; 2 d-tiles per PSUM tile
                    for mp in range(4):
                        ps_u = up_p.tile([128, 512], F32, tag="up")
                        for mi in range(2):
                            m = mp * 2 + mi
                            for kt in range(2):
                                nc.tensor.matmul(
                                    ps_u[:, mi * 256 : (mi + 1) * 256],
                                    lhsT=hk[:, kt, m * 128 : (m + 1) * 128],
                                    rhs=et[:, kt, :],
                                    start=(kt == 0),
                                    stop=(kt == 1),
                                )
                        v = v_p.tile([128, 2, G, NQ], F32, tag="v")
                        nc.vector.tensor_tensor(
                            out=v[:],
                            in0=ps_u[:].rearrange("p (m g q) -> p m g q", m=2, g=G),
                            in1=hqT[
                                :, mp * 2 : mp * 2 + 2, None, b * NQ : (b + 1) * NQ
                            ].to_broadcast([128, 2, G, NQ]),
                            op=OP.mult,
                        )
                        vr = v_p.tile([128, 2, G], F32, tag="vr")
                        nc.vector.tensor_reduce(
                            out=vr[:], in_=v[:], axis=AX.X, op=OP.add
                        )
                        with nc.allow_low_precision(reason="fp32r round"):
                            nc.vector.tensor_tensor(
                                out=poT[:, mp * 2 : mp * 2 + 2, :, b],
                                in0=vr[:],
                                in1=zbro[:, None, :].to_broadcast([128, 2, G]),
                                op=OP.mult,
                            )

            # attention pools are dead now; reuse their SBUF for the
            # remaining Wout/glove stream so phase F/G start with data resident
            actx.close()
            wout_p2 = ctx.enter_context(tc.tile_pool(name="wout2", bufs=NWOUT - 26))
            glove_p2 = ctx.enter_context(tc.tile_pool(name="glove2", bufs=NA_CH - 2))
            for t in range(26, NWOUT):
                wtile = wout_p2.tile([128, N_OUT], F32R, tag="wout2")
                nc.sync.dma_start(wtile[:], wout_d[t * 128 : (t + 1) * 128, :])
                wout_tiles.append(wtile)
            for a in range(2, NA_CH):
                gt = glove_p2.tile([128, 3, NA_W], F32R, tag="glove2")
                for c, rows in enumerate(N_CHUNKS):
                    nc.sync.dma_start(
                        gt[:rows, c, :],
                        glovet_d[c * 128 : c * 128 + rows,
                                 a * NA_W : (a + 1) * NA_W],
                    )
                glove_tiles.append(gt)

            # ---- phase F: out [8, 300] = pooled_flat @ Wout + bout ----
            ps_o = mm_p.tile([128, 512], F32, tag="mm")
            for g in range(G):
                for m in range(DT):
                    t = g * DT + m
                    nc.tensor.matmul(
                        ps_o[:BL, :N_OUT],
                        lhsT=poT[:, m, g, :],
                        rhs=wout_tiles[t][:],
                        start=(t == 0),
                        stop=(t == NWOUT - 1),
                    )
            out_sb = consts.tile([BL, N_OUT], F32R, tag="out_sb")
            nc.vector.tensor_tensor(
                out=out_sb[:], in0=ps_o[:BL, :N_OUT], in1=bout_sb[:], op=OP.add
            )

            # ---- phase G: sim + log_softmax ----
            outT = consts.tile([128, 3, BL], F32R, tag="outT")
            for c, rows in enumerate(N_CHUNKS):
                ps = mm_p.tile([128, 512], F32R, tag="mm")
                nc.tensor.transpose(
                    ps[:rows, :BL],
                    out_sb[:, c * 128 : c * 128 + rows],
                    ident[:BL, :BL],
                )
                nc.vector.tensor_copy(outT[:rows, c, :], ps[:rows, :BL])

            sim_sb = consts.tile([BL, N_ANS], F32, tag="sim_sb")
            esc = consts.tile([BL, NA_W], F32, tag="esc")
            mx8 = consts.tile([BL, NA_CH], F32, tag="mx8")
            zs8 = consts.tile([BL, NA_CH], F32, tag="zs8")
            mx = consts.tile([BL, 1], F32, tag="mx")
            nmx = consts.tile([BL, 1], F32, tag="nmx")
            zs = consts.tile([BL, 1], F32, tag="zs")
            lnz = consts.tile([BL, 1], F32, tag="lnz")
            for a in range(NA_CH):
                ps_s = mm_p.tile([128, 512], F32, tag="mm")
                for c, rows in enumerate(N_CHUNKS):
                    nc.tensor.matmul(
                        ps_s[:BL, :NA_W],
                        lhsT=outT[:rows, c, :],
                        rhs=glove_tiles[a][:rows, c, :],
                        start=(c == 0),
                        stop=(c == 2),
                    )
                nc.vector.tensor_reduce(
                    out=mx8[:, a : a + 1], in_=ps_s[:BL, :NA_W], axis=AX.X, op=OP.max
                )
                nc.vector.tensor_copy(sim_sb[:, a * NA_W : (a + 1) * NA_W],
                                      ps_s[:BL, :NA_W])
            nc.vector.tensor_reduce(out=mx[:], in_=mx8[:], axis=AX.X, op=OP.max)
            nc.vector.tensor_scalar_mul(nmx[:], mx[:], -1.0)
            for a in range(NA_CH):
                nc.scalar.activation(
                    out=esc[:],  # scratch, discarded
                    in_=sim_sb[:, a * NA_W : (a + 1) * NA_W],
                    func=AF.Exp,
                    bias=nmx[:],
                    accum_out=zs8[:, a : a + 1],
                )
            nc.vector.tensor_reduce(out=zs[:], in_=zs8[:], axis=AX.X, op=OP.add)
            nc.scalar.activation(out=lnz[:], in_=zs[:], func=AF.Ln)
            nc.vector.tensor_scalar(
                out=sim_sb[:],
                in0=sim_sb[:],
                scalar1=mx[:],
                scalar2=lnz[:],
                op0=OP.subtract,
                op1=OP.subtract,
            )
            nc.sync.dma_start(out_d, sim_sb[:])

    nc.compile()
    return nc


_NC = None


def _get_nc():
    global _NC
    if _NC is None:
        _NC = build_kernel()
    return _NC


def make_in_maps(inputs):
    he_q = np.asarray(inputs["he_ques"]).astype(np.int32)   # [64, 32]
    he_k = np.asarray(inputs["he_kg"]).astype(np.int32)     # [64, 256]
    emb0 = np.asarray(inputs["emb"], dtype=np.float32)
    emb = np.ones((VOCAB, EA), dtype=np.float32)            # ones col at E=300
    emb[:, :E] = emb0
    wq = np.concatenate(
        [np.asarray(inputs["Wq"], np.float32),
         np.asarray(inputs["bq"], np.float32)[None, :]], axis=0)
    wk = np.concatenate(
        [np.asarray(inputs["Wk"], np.float32),
         np.asarray(inputs["bk"], np.float32)[None, :]], axis=0)
    watt = np.ascontiguousarray(
        np.asarray(inputs["Watt"], np.float32).reshape(DT, 128, G)
        .transpose(1, 0, 2))                                # [128, DT, G]
    wout = np.ascontiguousarray(np.asarray(inputs["Wout"], np.float32))
    bout = np.ascontiguousarray(
        np.broadcast_to(np.asarray(inputs["bout"], np.float32), (BL, N_OUT)))
    glovet = np.ascontiguousarray(
        np.asarray(inputs["glove_cands"], np.float32).T)    # [300, 4000]
    ident = np.eye(128, dtype=np.float32)

    in_maps = []
    for i in range(N_CORES):
        iq = he_q[i * BL : (i + 1) * BL].reshape(-1)        # [256]
        ik = he_k[i * BL : (i + 1) * BL].reshape(-1)        # [2048]
        in_maps.append({
            "emb": emb,
            "idx_q": np.ascontiguousarray(iq.reshape(TQ_TILES, 128).T),
            "idx_k": np.ascontiguousarray(ik.reshape(TK_TILES, 128).T),
            "wq": wq,
            "wk": wk,
            "watt": watt,
            "wout": wout,
            "bout": bout,
            "glovet": glovet,
            "ident": ident,
            "ones_col": np.ones((128, 1), dtype=np.float32),
        })
    return in_maps


def kernel(**inputs) -> np.ndarray:
    nc = _get_nc()
    in_maps = make_in_maps(inputs)
    res = run_bass_kernel_spmd(nc, in_maps, list(range(N_CORES)))
    return np.concatenate([res.results[i]["out"] for i in range(N_CORES)], axis=0)



# revision 5
# speedup vs baseline: 1.1599x; 1.1599x over previous
"""Trainium2 Bass kernel for the BAN (bilinear attention network) problem.

Math (per batch b, eval mode):
    hq = emb[he_ques] @ Wq + bq                  [NQ, H]
    hk = emb[he_kg]   @ Wk + bk                  [NK, H]
    logits[g,q,k] = sum_d hq[q,d] Watt[d,g] hk[k,d]   (+ batt[g], cancels in
                                                       the joint softmax)
    att = softmax over flattened (q,k) per (b,g)
    pooled[g,d] = sum_{q,k} hq[q,d] att[g,q,k] hk[k,d]
    out = pooled.flat @ Wout + bout;  sim = out @ glove.T;  log_softmax(sim)

Distribution: pure data parallel over batch, 8 samples per core on 8 cores.
All weights replicated. No collectives.

v2 design (vs the 216us f32r baseline):
  - ALL matmul operands in bf16 (f32r measured ~0.9 ns/row on HW =
    fp32_mode=HIGH two-pass; bf16 is single-pass ~0.42 ns/row). PSUM
    accumulation stays fp32. Empirical scale-rel error ~1.7e-3 vs the
    2e-2 gate.
  - emb is host-cast to bf16 and padded to 384 columns (col 300 = ones for
    the bias trick, 301..383 = zeros) so the gathered X rows can be
    transposed by the DMA XBAR (dma_start_transpose needs free%128==0).
  - All X / hk transposes moved off the PE onto XBAR transpose DMAs;
    the token-major hk (needed as lhsT for the pooling matmul) comes from
    transposing hkT instead of a second projection matmul pass.
  - Identity built on-chip (memset + affine_select) so the PE warmup needs
    no DMA round-trip; warmup = back-to-back N=512 bf16 matmuls to push the
    HAM clock ramp while initial DMAs stream.
  - Weight streams consolidated into single big dma_starts (packets spread
    across all 16 DMA engines automatically; saves ~600ns sequencer issue
    cost per DMA).
  - Tail: no max subtraction (sim in [-4.5, 4.3] so exp is safe), sim kept
    resident in PSUM across phase G, exp/reduce pipelined per chunk, final
    (sim - lnZ) split across DVE and Act, ln(Z) computed as Ln(1/Z) to
    avoid a negate.
"""

import sys

if "/opt/trn_rl_repo" not in sys.path:
    sys.path.insert(0, "/opt/trn_rl_repo")

import numpy as np

import concourse.bass as bass
import concourse.mybir as mybir
import concourse.tile as tile
from concourse import bacc
from concourse.bass_utils import run_bass_kernel_spmd

F32 = mybir.dt.float32
BF16 = mybir.dt.bfloat16
I32 = mybir.dt.int32
AX = mybir.AxisListType
OP = mybir.AluOpType
AF = mybir.ActivationFunctionType

N_CORES = 8
VOCAB = 20000
E = 300          # word embedding size
EA = 384         # padded: col 300 = ones (bias trick), 301.. = zeros
H = 1024         # hidden
G = 8            # heads
N_OUT = 300
N_ANS = 4000
B, NQ, NK = 64, 32, 256
BL = B // N_CORES            # 8 samples per core
TQ = BL * NQ                 # 256 q tokens per core
TK = BL * NK                 # 2048 k tokens per core
TQ_TILES = TQ // 128         # 2
TK_TILES = TK // 128         # 16
DT = H // 128                # 8 d-tiles
N_CHUNKS = (128, 128, N_OUT - 256)   # (128, 128, 44) rows of the 300-dim
NA_CH = 8                    # sim computed in 8 chunks of 500
NA_W = N_ANS // NA_CH        # 500
NWOUT = G * DT               # 64 Wout k-tiles


def build_kernel():
    nc = bacc.Bacc("TRN2", target_bir_lowering=False, debug=False,
                   num_devices=N_CORES)

    # ---- DRAM I/O ----
    emb_d = nc.dram_tensor("emb", [VOCAB, EA], BF16, kind="ExternalInput").ap()
    idxq_d = nc.dram_tensor("idx_q", [128, TQ_TILES], I32, kind="ExternalInput").ap()
    idxk_d = nc.dram_tensor("idx_k", [128, TK_TILES], I32, kind="ExternalInput").ap()
    wq_d = nc.dram_tensor("wq", [EA, H], BF16, kind="ExternalInput").ap()
    wk_d = nc.dram_tensor("wk", [EA, H], BF16, kind="ExternalInput").ap()
    watt_d = nc.dram_tensor("watt", [128, DT, G], BF16, kind="ExternalInput").ap()
    wout_d = nc.dram_tensor("wout", [G * H, N_OUT], BF16, kind="ExternalInput").ap()
    bout_d = nc.dram_tensor("bout", [BL, N_OUT], F32, kind="ExternalInput").ap()
    glovet_d = nc.dram_tensor("glovet", [N_OUT, N_ANS], BF16,
                              kind="ExternalInput").ap()
    out_d = nc.dram_tensor("out", [BL, N_ANS], F32, kind="ExternalOutput").ap()
    warm_d = nc.dram_tensor("warm", [1, 128], F32, kind="ExternalOutput").ap()

    with tile.TileContext(nc) as tc:
        import contextlib

        with contextlib.ExitStack() as ctx:
            consts = ctx.enter_context(tc.tile_pool(name="consts", bufs=1))
            actx = contextlib.ExitStack()
            xrow_p = actx.enter_context(tc.tile_pool(name="xrow", bufs=4))
            xkt_p = actx.enter_context(tc.tile_pool(name="xkt", bufs=2))
            hkt_p = actx.enter_context(tc.tile_pool(name="hkt", bufs=2))
            hk_p = actx.enter_context(tc.tile_pool(name="hk", bufs=2))
            hqw_p = actx.enter_context(tc.tile_pool(name="hqw", bufs=2))
            et_p = actx.enter_context(tc.tile_pool(name="et", bufs=2))
            v_p = actx.enter_context(tc.tile_pool(name="v", bufs=2))
            zz_p = actx.enter_context(tc.tile_pool(name="zz", bufs=2))
            zn_p = actx.enter_context(tc.tile_pool(name="zn", bufs=2))
            mm_p = actx.enter_context(tc.tile_pool(name="mm", bufs=4, space="PSUM"))
            lg_p = actx.enter_context(tc.tile_pool(name="lg", bufs=2, space="PSUM"))
            up_p = actx.enter_context(tc.tile_pool(name="up", bufs=2, space="PSUM"))

            # ---- on-chip constants (no DMA needed) ----
            ident = consts.tile([128, 128], BF16, tag="ident")
            nc.gpsimd.memset(ident[:], 1.0)
            nc.gpsimd.affine_select(
                out=ident[:], in_=ident[:], pattern=[[-1, 128]], base=0,
                channel_multiplier=1, compare_op=OP.is_equal, fill=0.0,
            )
            ones_sb = consts.tile([128, 1], F32, tag="ones")
            nc.gpsimd.memset(ones_sb[:], 1.0)
            wz = consts.tile([128, 512], BF16, tag="wz")
            nc.vector.memset(wz[:], 0.0)

            # PE warmup: back-to-back N=512 bf16 matmuls to push the HAM
            # clock ramp while the initial DMAs stream in.
            wps = mm_p.tile([128, 512], F32, tag="mm")
            for _ in range(20):
                nc.tensor.matmul(wps[:], lhsT=ident[:], rhs=wz[:],
                                 start=True, stop=True)
            warm_sb = consts.tile([1, 128], F32, tag="warm")
            nc.vector.tensor_copy(warm_sb[:], wps[:1, :128])
            nc.sync.dma_start(warm_d, warm_sb[:])

            # ---- input DMAs (SP sequencer; one big dma_start each) ----
            idxq_sb = consts.tile([128, TQ_TILES], I32, tag="idxq")
            nc.sync.dma_start(idxq_sb[:], idxq_d)
            idxk_sb = consts.tile([128, TK_TILES], I32, tag="idxk")
            nc.sync.dma_start(idxk_sb[:], idxk_d)
            wq_sb = consts.tile([128, 3, H], BF16, tag="wq")
            nc.sync.dma_start(
                wq_sb[:], wq_d.rearrange("(c p) h -> p c h", p=128))
            wk_sb = consts.tile([128, 3, H], BF16, tag="wk")
            nc.sync.dma_start(
                wk_sb[:], wk_d.rearrange("(c p) h -> p c h", p=128))
            watt_sb = consts.tile([128, DT, G], BF16, tag="watt")
            nc.sync.dma_start(watt_sb[:], watt_d)
            bout_sb = consts.tile([BL, N_OUT], F32, tag="bout")
            nc.sync.dma_start(bout_sb[:], bout_d)

            # streamed weights for phases F/G (issued early, land mid-kernel)
            wout_sb = consts.tile([128, NWOUT, N_OUT], BF16, tag="wout")
            nc.sync.dma_start(
                wout_sb[:], wout_d.rearrange("(t p) n -> p t n", p=128))
            glove_sb = consts.tile([128, 3, N_ANS], BF16, tag="glove")
            nc.sync.dma_start(
                glove_sb[:, :2, :],
                glovet_d[: 2 * 128].rearrange("(c p) n -> p c n", p=128))
            nc.sync.dma_start(glove_sb[: N_OUT - 256, 2, :],
                              glovet_d[2 * 128 : N_OUT])

            def gather_transpose(idx_sb, col, dst, dst_col):
                """Gather 128 emb rows (bf16, EA=384 cols) via SWDGE, then
                XBAR-transpose into dst[:, c, dst_col*128 : ...] for the 3
                column chunks. The ones column (E index 300) lands at
                partition 44 of chunk 2."""
                xrow = xrow_p.tile([128, EA], BF16, tag="xrow")
                nc.gpsimd.indirect_dma_start(
                    out=xrow[:],
                    out_offset=None,
                    in_=emb_d,
                    in_offset=bass.IndirectOffsetOnAxis(
                        ap=idx_sb[:, col : col + 1], axis=0
                    ),
                )
                nc.scalar.dma_start_transpose(
                    dst[:, :, dst_col * 128 : (dst_col + 1) * 128], xrow[:]
                )

            # ---- phase B: gather+transpose Xq -> xqT [128, 3, TQ] ----
            xqT = consts.tile([128, 3, TQ], BF16, tag="xqT")
            for t in range(TQ_TILES):
                gather_transpose(idxq_sb, t, xqT, t)

            # ---- phase C: hqT [128, DT, TQ] bf16 ----
            hqT = consts.tile([128, DT, TQ], BF16, tag="hqT")
            for m in range(DT):
                ps = mm_p.tile([128, 512], F32, tag="mm")
                for c in range(3):
                    nc.tensor.matmul(
                        ps[:, :TQ],
                        lhsT=wq_sb[:, c, m * 128 : (m + 1) * 128],
                        rhs=xqT[:, c, :],
                        start=(c == 0),
                        stop=(c == 2),
                    )
                nc.scalar.activation(out=hqT[:, m, :], in_=ps[:, :TQ],
                                     func=AF.Copy)

            poT = consts.tile([128, DT, G, BL], BF16, tag="poT")

            # ---- phase D: attention, two samples per pair ----
            for p in range(BL // 2):
                # D1: gather + XBAR-transpose K tokens for samples 2p, 2p+1
                xkT = xkt_p.tile([128, 3, 512], BF16, tag="xkT")
                for t in range(4):
                    gather_transpose(idxk_sb, p * 4 + t, xkT, t)

                # D2: hkT for the pair [128, DT, 512] bf16
                hkT = hkt_p.tile([128, DT, 512], BF16, tag="hkT")
                for m in range(DT):
                    ps = mm_p.tile([128, 512], F32, tag="mm")
                    for c in range(3):
                        nc.tensor.matmul(
                            ps[:],
                            lhsT=wk_sb[:, c, m * 128 : (m + 1) * 128],
                            rhs=xkT[:, c, :],
                            start=(c == 0),
                            stop=(c == 2),
                        )
                    if m % 2 == 0:
                        nc.scalar.activation(out=hkT[:, m, :], in_=ps[:],
                                             func=AF.Copy)
                    else:
                        nc.vector.tensor_copy(out=hkT[:, m, :], in_=ps[:])

                # D2b: token-major hk for the pair via XBAR transpose:
                # hk[k, t, d-chunk m] with k = pair-local token, t in 0..3
                hk = hk_p.tile([128, 4, H], BF16, tag="hk")
                for m in range(DT):
                    nc.sync.dma_start_transpose(
                        hk[:, :, m * 128 : (m + 1) * 128], hkT[:, m, :]
                    )

                for bi in range(2):
                    b = p * 2 + bi

                    # D4: hqw [128, DT, G, NQ] = hqT(b) * watt (bcast over g)
                    hqw = hqw_p.tile([128, DT, G, NQ], BF16, tag="hqw")
                    nc.vector.tensor_tensor(
                        out=hqw[:],
                        in0=hqT[:, :, None, b * NQ : (b + 1) * NQ].to_broadcast(
                            [128, DT, G, NQ]
                        ),
                        in1=watt_sb[:, :, :, None].to_broadcast([128, DT, G, NQ]),
                        op=OP.mult,
                    )

                    # D5: logits.T [k, (g,q)] in PSUM: [128, 2, 256]
                    ps_l = lg_p.tile([128, 512], F32, tag="lg")
                    for kt in range(2):
                        for c in range(DT):
                            nc.tensor.matmul(
                                ps_l[:, kt * 256 : (kt + 1) * 256],
                                lhsT=hkT[
                                    :, c,
                                    bi * 256 + kt * 128 : bi * 256 + (kt + 1) * 128,
                                ],
                                rhs=hqw[:, c],
                                start=(c == 0),
                                stop=(c == DT - 1),
                            )

                    # D6: E = exp(logits) bf16, per-g row sums zz (f32)
                    et = et_p.tile([128, 2, G * NQ], BF16, tag="et")
                    zz = zz_p.tile([128, 2, G], F32, tag="zz")
                    for kt in range(2):
                        nc.scalar.activation(
                            out=et[:, kt, :],
                            in_=ps_l[:, kt * 256 : (kt + 1) * 256],
                            func=AF.Exp,
                        )
                        nc.vector.tensor_reduce(
                            out=zz[:, kt, :],
                            in_=et[:, kt].rearrange("p (g q) -> p g q", g=G),
                            axis=AX.X,
                            op=OP.add,
                        )

                    # D7: Z_g = sum over k-partitions; zinv broadcast to 128
                    ps_z = mm_p.tile([128, 512], F32, tag="mm")
                    for kt in range(2):
                        nc.tensor.matmul(
                            ps_z[:1, :G],
                            lhsT=ones_sb[:],
                            rhs=zz[:, kt, :],
                            start=(kt == 0),
                            stop=(kt == 1),
                        )
                    zinv = zn_p.tile([1, G], F32, tag="zinv")
                    nc.vector.reciprocal(zinv[:1, :], ps_z[:1, :G])
                    zbro = zn_p.tile([128, G], F32, tag="zbro")
                    nc.gpsimd.partition_broadcast(zbro[:], zinv[:1, :], channels=128)

                    # D8: u.T, v, pooled partial sums; 2 d-tiles per PSUM tile
                    for mp in range(4):
                        ps_u = up_p.tile([128, 512], F32, tag="up")
                        for mi in range(2):
                            m = mp * 2 + mi
                            for kt in range(2):
                                nc.tensor.matmul(
                                    ps_u[:, mi * 256 : (mi + 1) * 256],
                                    lhsT=hk[:, bi * 2 + kt, m * 128 : (m + 1) * 128],
                                    rhs=et[:, kt, :],
                                    start=(kt == 0),
                                    stop=(kt == 1),
                                )
                        v = v_p.tile([128, 2, G, NQ], BF16, tag="v")
                        nc.vector.tensor_tensor(
                            out=v[:],
                            in0=ps_u[:].rearrange("p (m g q) -> p m g q", m=2, g=G),
                            in1=hqT[
                                :, mp * 2 : mp * 2 + 2, None, b * NQ : (b + 1) * NQ
                            ].to_broadcast([128, 2, G, NQ]),
                            op=OP.mult,
                        )
                        vr = v_p.tile([128, 2, G], F32, tag="vr")
                        nc.vector.tensor_reduce(
                            out=vr[:], in_=v[:], axis=AX.X, op=OP.add
                        )
                        with nc.allow_low_precision(reason="bf16 pooled"):
                            nc.vector.tensor_tensor(
                                out=poT[:, mp * 2 : mp * 2 + 2, :, b],
                                in0=vr[:],
                                in1=zbro[:, None, :].to_broadcast([128, 2, G]),
                                op=OP.mult,
                            )

            # attention pools (incl. all PSUM) are dead now
            actx.close()
            fctx = contextlib.ExitStack()
            fo_p = fctx.enter_context(tc.tile_pool(name="fo", bufs=2, space="PSUM"))

            # ---- phase F: out [8, 300] = pooled_flat @ Wout + bout ----
            ps_o = fo_p.tile([128, 512], F32, tag="fo")
            for g in range(G):
                for m in range(DT):
                    t = g * DT + m
                    nc.tensor.matmul(
                        ps_o[:BL, :N_OUT],
                        lhsT=poT[:, m, g, :],
                        rhs=wout_sb[:, t, :],
                        start=(t == 0),
                        stop=(t == NWOUT - 1),
                    )
            out_sb = consts.tile([BL, N_OUT], BF16, tag="out_sb")
            with nc.allow_low_precision(reason="bf16 out"):
                nc.vector.tensor_tensor(
                    out=out_sb[:], in0=ps_o[:BL, :N_OUT], in1=bout_sb[:], op=OP.add
                )

            # ---- phase G: sim + log_softmax (no max shift; sim is O(+-5)) --
            outT = consts.tile([128, 3, BL], BF16, tag="outT")
            for c, rows in enumerate(N_CHUNKS):
                psT = fo_p.tile([128, 128], BF16, tag="foT")
                nc.tensor.transpose(
                    psT[:rows, :BL],
                    out_sb[:, c * 128 : c * 128 + rows],
                    ident[:BL, :BL],
                )
                nc.scalar.activation(out=outT[:rows, c, :], in_=psT[:rows, :BL],
                                     func=AF.Copy)

            zs8 = consts.tile([BL, NA_CH], F32, tag="zs8")
            zs = consts.tile([BL, 1], F32, tag="zs")
            zsi = consts.tile([BL, 1], F32, tag="zsi")
            nlnz = consts.tile([BL, 1], F32, tag="nlnz")
            final_sb = consts.tile([BL, N_ANS], F32, tag="final")

            simp_tiles = []
            esc_p = ctx.enter_context(tc.tile_pool(name="esc", bufs=2))
            fctx.close()  # free F/outT PSUM banks before claiming all 8
            sim_p = ctx.enter_context(tc.tile_pool(name="simp", bufs=NA_CH,
                                                   space="PSUM"))
            for a in range(NA_CH):
                ps_s = sim_p.tile([128, NA_W], F32, tag="simp")
                for c, rows in enumerate(N_CHUNKS):
                    nc.tensor.matmul(
                        ps_s[:BL, :],
                        lhsT=outT[:rows, c, :],
                        rhs=glove_sb[:rows, c, a * NA_W : (a + 1) * NA_W],
                        start=(c == 0),
                        stop=(c == 2),
                    )
                esc = esc_p.tile([BL, NA_W], BF16, tag="esc")
                nc.scalar.activation(out=esc[:], in_=ps_s[:BL, :], func=AF.Exp)
                nc.vector.tensor_reduce(
                    out=zs8[:, a : a + 1], in_=esc[:], axis=AX.X, op=OP.add
                )
                simp_tiles.append(ps_s)

            nc.vector.tensor_reduce(out=zs[:], in_=zs8[:], axis=AX.X, op=OP.add)
            nc.vector.reciprocal(zsi[:], zs[:])
            nc.scalar.activation(out=nlnz[:], in_=zsi[:], func=AF.Ln)
            # final = sim - lnZ, 4 chunks on DVE + 4 on Act, then 2 DMAs
            for a in range(NA_CH):
                span = slice(a * NA_W, (a + 1) * NA_W)
                if a % 2 == 0:
                    nc.vector.tensor_scalar(
                        out=final_sb[:, span], in0=simp_tiles[a][:BL, :],
                        scalar1=nlnz[:], scalar2=None,
                        op0=OP.add,
                    )
                else:
                    nc.scalar.activation(
                        out=final_sb[:, span], in_=simp_tiles[a][:BL, :],
                        func=AF.Identity, bias=nlnz[:],
                    )
                if a == 3:
                    nc.sync.dma_start(out_d[:, : 4 * NA_W], final_sb[:, : 4 * NA_W])
            nc.sync.dma_start(out_d[:, 4 * NA_W :], final_sb[:, 4 * NA_W :])

    nc.compile()
    return nc


_NC = None


def _get_nc():
    global _NC
    if _NC is None:
        _NC = build_kernel()
    return _NC


def make_in_maps(inputs):
    import ml_dtypes

    bf = ml_dtypes.bfloat16
    he_q = np.asarray(inputs["he_ques"]).astype(np.int32)   # [64, 32]
    he_k = np.asarray(inputs["he_kg"]).astype(np.int32)     # [64, 256]
    emb0 = np.asarray(inputs["emb"], dtype=np.float32)
    emb = np.zeros((VOCAB, EA), dtype=bf)
    emb[:, :E] = emb0.astype(bf)
    emb[:, E] = np.ones((), dtype=bf)                       # bias column
    wq = np.zeros((EA, H), dtype=bf)
    wq[:E] = np.asarray(inputs["Wq"], np.float32).astype(bf)
    wq[E] = np.asarray(inputs["bq"], np.float32).astype(bf)
    wk = np.zeros((EA, H), dtype=bf)
    wk[:E] = np.asarray(inputs["Wk"], np.float32).astype(bf)
    wk[E] = np.asarray(inputs["bk"], np.float32).astype(bf)
    watt = np.ascontiguousarray(
        np.asarray(inputs["Watt"], np.float32).reshape(DT, 128, G)
        .transpose(1, 0, 2)).astype(bf)                     # [128, DT, G]
    wout = np.ascontiguousarray(
        np.asarray(inputs["Wout"], np.float32)).astype(bf)
    bout = np.ascontiguousarray(
        np.broadcast_to(np.asarray(inputs["bout"], np.float32), (BL, N_OUT)))
    glovet = np.ascontiguousarray(
        np.asarray(inputs["glove_cands"], np.float32).T).astype(bf)  # [300,4000]

    in_maps = []
    for i in range(N_CORES):
        iq = he_q[i * BL : (i + 1) * BL].reshape(-1)        # [256]
        ik = he_k[i * BL : (i + 1) * BL].reshape(-1)        # [2048]
        in_maps.append({
            "emb": emb,
            "idx_q": np.ascontiguousarray(iq.reshape(TQ_TILES, 128).T),
            "idx_k": np.ascontiguousarray(ik.reshape(TK_TILES, 128).T),
            "wq": wq,
            "wk": wk,
            "watt": watt,
            "wout": wout,
            "bout": bout,
            "glovet": glovet,
        })
    return in_maps


def kernel(**inputs) -> np.ndarray:
    nc = _get_nc()
    in_maps = make_in_maps(inputs)
    res = run_bass_kernel_spmd(nc, in_maps, list(range(N_CORES)))
    return np.concatenate(
        [np.asarray(res.results[i]["out"], np.float32) for i in range(N_CORES)],
        axis=0,
    )


# revision 13
# speedup vs baseline: 1.2764x; 1.1004x over previous
"""Trainium2 Bass kernel for the BAN (bilinear attention network) problem.

Math (per batch b, eval mode):
    hq = emb[he_ques] @ Wq + bq                  [NQ, H]
    hk = emb[he_kg]   @ Wk + bk                  [NK, H]
    logits[g,q,k] = sum_d hq[q,d] Watt[d,g] hk[k,d]   (+ batt[g], cancels in
                                                       the joint softmax)
    att = softmax over flattened (q,k) per (b,g)
    pooled[g,d] = sum_{q,k} hq[q,d] att[g,q,k] hk[k,d]
    out = pooled.flat @ Wout + bout;  sim = out @ glove.T;  log_softmax(sim)

Distribution: pure data parallel over batch, 8 samples per core on 8 cores.
All weights replicated. No collectives.

v5 design notes:
  - ALL matmul operands bf16 (single-pass PE ~0.42ns/row vs f32r two-pass).
    PSUM accumulation fp32. Scale-rel error ~1.7e-3 vs the 2e-2 gate.
  - emb host-cast bf16, padded to 384 cols (col 300 = ones bias trick);
    gathered X rows XBAR-transposed (one DMA per gather tile, Sync queue).
  - hkT computed by matmul (the D5 critical path never depends on a DMA
    transpose); token-major hk derived on the PE (transpose+copy, ~1us a
    pair) because each XBAR transpose costs ~1.3us of hwdge sequencer
    time, and sharing DMA completion semaphores across queues with the
    big weight streams caused false-satisfied waits (a real data race).
  - hqw (hq * Watt) batched for all samples inside phase C, layout
    [d, m, b, g, q] so D5's rhs slice is contiguous.
  - Z-reduction (D7) runs after the pooling matmuls so its tiny dependent
    matmul never stalls the PE; single fused pooled scale per sample.
  - wout/glove streams split into chunks emitted across the pair loop:
    one 4.9MB DMA monopolized the DMA path for ~18us and serialized
    unrelated transfers behind it.
  - Startup: identity built on-chip; K-pair-0 gathers before Q gathers;
    PE order warm -> D2(pair0) -> C -> samples.
  - Tail: no max shift (sim is O(+-5)); sim resident in PSUM; exp/reduce
    pipelined per chunk; final (sim - lnZ) split DVE/Act; lnZ via Ln(1/Z).
"""

import sys

if "/opt/trn_rl_repo" not in sys.path:
    sys.path.insert(0, "/opt/trn_rl_repo")

import numpy as np

import concourse.bass as bass
import concourse.mybir as mybir
import concourse.tile as tile
from concourse import bacc
from concourse.bass_utils import run_bass_kernel_spmd

F32 = mybir.dt.float32
BF16 = mybir.dt.bfloat16
I32 = mybir.dt.int32
AX = mybir.AxisListType
OP = mybir.AluOpType
AF = mybir.ActivationFunctionType

N_CORES = 8
VOCAB = 20000
E = 300          # word embedding size
EA = 384         # padded: col 300 = ones (bias trick), 301.. = zeros
H = 1024         # hidden
G = 8            # heads
N_OUT = 300
N_ANS = 4000
B, NQ, NK = 64, 32, 256
BL = B // N_CORES            # 8 samples per core
TQ = BL * NQ                 # 256 q tokens per core
TK = BL * NK                 # 2048 k tokens per core
TQ_TILES = TQ // 128         # 2
TK_TILES = TK // 128         # 16
DT = H // 128                # 8 d-tiles
N_CHUNKS = (128, 128, N_OUT - 256)   # (128, 128, 44) rows of the 300-dim
NA_CH = 8                    # sim computed in 8 chunks of 500
NA_W = N_ANS // NA_CH        # 500
NWOUT = G * DT               # 64 Wout k-tiles


def build_kernel():
    nc = bacc.Bacc("TRN2", target_bir_lowering=False, debug=False,
                   num_devices=N_CORES)

    # ---- DRAM I/O ----
    emb_d = nc.dram_tensor("emb", [VOCAB, EA], BF16, kind="ExternalInput").ap()
    idxq_d = nc.dram_tensor("idx_q", [128, TQ_TILES], I32, kind="ExternalInput").ap()
    idxk_d = nc.dram_tensor("idx_k", [128, TK_TILES], I32, kind="ExternalInput").ap()
    wq_d = nc.dram_tensor("wq", [EA, H], BF16, kind="ExternalInput").ap()
    wk_d = nc.dram_tensor("wk", [EA, H], BF16, kind="ExternalInput").ap()
    watt_d = nc.dram_tensor("watt", [128, DT, G], BF16, kind="ExternalInput").ap()
    wout_d = nc.dram_tensor("wout", [G * H, N_OUT], BF16, kind="ExternalInput").ap()
    bout_d = nc.dram_tensor("bout", [BL, N_OUT], F32, kind="ExternalInput").ap()
    glovet_d = nc.dram_tensor("glovet", [N_OUT, N_ANS], BF16,
                              kind="ExternalInput").ap()
    out_d = nc.dram_tensor("out", [BL, N_ANS], F32, kind="ExternalOutput").ap()
    warm_d = nc.dram_tensor("warm", [1, 128], F32, kind="ExternalOutput").ap()

    with tile.TileContext(nc) as tc:
        import contextlib

        with contextlib.ExitStack() as ctx:
            consts = ctx.enter_context(tc.tile_pool(name="consts", bufs=1))
            actx = contextlib.ExitStack()
            hqw_pool = actx.enter_context(tc.tile_pool(name="hqwp", bufs=1))
            xrow_p = actx.enter_context(tc.tile_pool(name="xrow", bufs=6))
            xkt_p = actx.enter_context(tc.tile_pool(name="xkt", bufs=2))
            hkt_p = actx.enter_context(tc.tile_pool(name="hkt", bufs=2))
            hk_p = actx.enter_context(tc.tile_pool(name="hk", bufs=2))
            et_p = actx.enter_context(tc.tile_pool(name="et", bufs=2))
            v_p = actx.enter_context(tc.tile_pool(name="v", bufs=2))
            vr_p = actx.enter_context(tc.tile_pool(name="vr", bufs=3))
            zz_p = actx.enter_context(tc.tile_pool(name="zz", bufs=2))
            zn_p = actx.enter_context(tc.tile_pool(name="zn", bufs=3))
            mm_p = actx.enter_context(tc.tile_pool(name="mm", bufs=2, space="PSUM"))
            tp_p = actx.enter_context(tc.tile_pool(name="tp", bufs=2, space="PSUM"))
            lg_p = actx.enter_context(tc.tile_pool(name="lg", bufs=2, space="PSUM"))
            up_p = actx.enter_context(tc.tile_pool(name="up", bufs=2, space="PSUM"))

            # ---- on-chip constants (no DMA round-trip) ----
            ident = consts.tile([128, 128], BF16, tag="ident")
            nc.gpsimd.memset(ident[:], 1.0)
            nc.gpsimd.affine_select(
                out=ident[:], in_=ident[:], pattern=[[-1, 128]], base=0,
                channel_multiplier=1, compare_op=OP.is_equal, fill=0.0,
            )
            ones_sb = consts.tile([128, 1], F32, tag="ones")
            nc.gpsimd.memset(ones_sb[:], 1.0)
            wz = consts.tile([128, 512], BF16, tag="wz")
            nc.vector.memset(wz[:], 0.0)

            # ---- critical input DMAs ----
            idxq_sb = consts.tile([128, TQ_TILES], I32, tag="idxq")
            nc.sync.dma_start(idxq_sb[:], idxq_d)
            idxk_sb = consts.tile([128, TK_TILES], I32, tag="idxk")
            nc.sync.dma_start(idxk_sb[:], idxk_d)
            wq_sb = consts.tile([128, 3, H], BF16, tag="wq")
            nc.sync.dma_start(
                wq_sb[:], wq_d.rearrange("(c p) h -> p c h", p=128))
            wk_sb = consts.tile([128, 3, H], BF16, tag="wk")
            nc.sync.dma_start(
                wk_sb[:], wk_d.rearrange("(c p) h -> p c h", p=128))
            watt_sb = consts.tile([128, DT, G], BF16, tag="watt")
            nc.sync.dma_start(watt_sb[:], watt_d)

            # ---- gathers: K pair 0 first (longer downstream chain) ----
            xrow_tiles = {}

            def gather(idx_sb, col):
                xrow = xrow_p.tile([128, EA], BF16, tag="xrow")
                nc.gpsimd.indirect_dma_start(
                    out=xrow[:],
                    out_offset=None,
                    in_=emb_d,
                    in_offset=bass.IndirectOffsetOnAxis(
                        ap=idx_sb[:, col : col + 1], axis=0
                    ),
                )
                return xrow

            def transpose_x(xrow, dst, dst_col):
                nc.sync.dma_start_transpose(
                    dst[:, :, dst_col * 128 : (dst_col + 1) * 128], xrow[:]
                )

            xqT = consts.tile([128, 3, TQ], BF16, tag="xqT")
            xkT0 = xkt_p.tile([128, 3, 512], BF16, tag="xkT")
            for t in range(4):
                xrow_tiles[("k", t)] = gather(idxk_sb, t)
            for t in range(TQ_TILES):
                xrow_tiles[("q", t)] = gather(idxq_sb, t)
            for t in range(4):
                transpose_x(xrow_tiles.pop(("k", t)), xkT0, t)
            for t in range(TQ_TILES):
                transpose_x(xrow_tiles.pop(("q", t)), xqT, t)

            # PE warmup: back-to-back N=512 bf16 matmuls push the HAM clock
            # ramp while the gathers land.
            wps = mm_p.tile([128, 512], F32, tag="mm")
            for _ in range(20):
                nc.tensor.matmul(wps[:], lhsT=ident[:], rhs=wz[:],
                                 start=True, stop=True)
            warm_sb = consts.tile([1, 128], F32, tag="warm")
            nc.vector.tensor_copy(warm_sb[:], wps[:1, :128])
            nc.sync.dma_start(warm_d, warm_sb[:])

            def project_k_pair(xkT):
                """hkT [d, k-pair] by matmul (so D5 never waits on a DMA
                transpose), then token-major hk via PE transposes, lagged
                one d-tile so each hkT copy has landed before its
                transpose reads it."""
                hkT = hkt_p.tile([128, DT, 512], BF16, tag="hkT")
                hk = hk_p.tile([128, 4, H], BF16, tag="hk")

                def emit_d2(m):
                    ps = mm_p.tile([128, 512], F32, tag="mm")
                    for c in range(3):
                        nc.tensor.matmul(
                            ps[:],
                            lhsT=wk_sb[:, c, m * 128 : (m + 1) * 128],
                            rhs=xkT[:, c, :],
                            start=(c == 0),
                            stop=(c == 2),
                        )
                    if m % 2 == 0:
                        nc.scalar.activation(out=hkT[:, m, :], in_=ps[:],
                                             func=AF.Copy)
                    else:
                        nc.vector.tensor_copy(out=hkT[:, m, :], in_=ps[:])

                def emit_transpose(m):
                    pt = tp_p.tile([128, 512], BF16, tag="mmT")
                    for t in range(4):
                        nc.tensor.transpose(
                            pt[:, t * 128 : (t + 1) * 128],
                            hkT[:, m, t * 128 : (t + 1) * 128],
                            ident[:],
                        )
                    nc.scalar.activation(
                        out=hk[:, :, m * 128 : (m + 1) * 128],
                        in_=pt[:].rearrange("p (t f) -> p t f", t=4),
                        func=AF.Copy)

                emit_d2(0)
                for m in range(1, DT):
                    emit_d2(m)
                    emit_transpose(m - 1)
                emit_transpose(DT - 1)
                return hk, hkT

            # ---- D2 pair 0 (before phase C so the PE rides the K path) ----
            hk_cur, hkT_cur = project_k_pair(xkT0)

            # ---- phase C: hqT + batched hqw, interleaved per d-tile ----
            # hqw layout [d, m, b, g, q]: D5's rhs slice [c, b] contiguous.
            hqT = consts.tile([128, DT, TQ], BF16, tag="hqT")
            hqw = hqw_pool.tile([128, DT, BL, G, NQ], BF16, tag="hqw")
            for m in range(DT):
                ps = mm_p.tile([128, 512], F32, tag="mm")
                for c in range(3):
                    nc.tensor.matmul(
                        ps[:, :TQ],
                        lhsT=wq_sb[:, c, m * 128 : (m + 1) * 128],
                        rhs=xqT[:, c, :],
                        start=(c == 0),
                        stop=(c == 2),
                    )
                nc.scalar.activation(out=hqT[:, m, :], in_=ps[:, :TQ],
                                     func=AF.Copy)
                nc.vector.tensor_tensor(
                    out=hqw[:, m],
                    in0=hqT[:, m, :].rearrange("p (b q) -> p b q", b=BL)[
                        :, :, None, :].to_broadcast([128, BL, G, NQ]),
                    in1=watt_sb[:, m, None, :, None].to_broadcast(
                        [128, BL, G, NQ]),
                    op=OP.mult,
                )

            # deferred weight streams, chunked so no single transfer
            # monopolizes the DMA path or skews semaphore completion order
            wout_sb = consts.tile([128, NWOUT, N_OUT], BF16, tag="wout")
            glove_sb = consts.tile([128, 3, N_ANS], BF16, tag="glove")
            bout_sb = consts.tile([BL, N_OUT], F32, tag="bout")

            def emit_weight_chunk(step):
                if step < 8:        # wout: 8 chunks of 8 k-tiles
                    lo = step * 8
                    nc.sync.dma_start(
                        wout_sb[:, lo : lo + 8, :],
                        wout_d[lo * 128 : (lo + 8) * 128].rearrange(
                            "(t p) n -> p t n", p=128))
                elif step < 10:     # glove rows 0..255 in 2 chunks
                    c = step - 8
                    nc.sync.dma_start(
                        glove_sb[:, c, :],
                        glovet_d[c * 128 : (c + 1) * 128])
                elif step == 10:    # glove rows 256..299
                    nc.sync.dma_start(glove_sb[: N_OUT - 256, 2, :],
                                      glovet_d[2 * 128 : N_OUT])
                elif step == 11:
                    nc.sync.dma_start(bout_sb[:], bout_d)

            poT = consts.tile([128, DT, G, BL], BF16, tag="poT")
            wstep = 0

            # ---- phase D: attention, two samples per pair ----
            for p in range(BL // 2):
                hk, hkT = hk_cur, hkT_cur
                xkT_next = None
                if p < 3:
                    xkT_next = xkt_p.tile([128, 3, 512], BF16, tag="xkT")
                    for t in range(2):
                        xrow_tiles[("k", t)] = gather(idxk_sb, (p + 1) * 4 + t)

                for bi in range(2):
                    b = p * 2 + bi

                    # D5: logits.T [k, (g,q)] in PSUM: [128, 2, 256]
                    ps_l = lg_p.tile([128, 512], F32, tag="lg")
                    for kt in range(2):
                        for c in range(DT):
                            nc.tensor.matmul(
                                ps_l[:, kt * 256 : (kt + 1) * 256],
                                lhsT=hkT[
                                    :, c,
                                    bi * 256 + kt * 128 : bi * 256 + (kt + 1) * 128,
                                ],
                                rhs=hqw[:, c, b],
                                start=(c == 0),
                                stop=(c == DT - 1),
                            )

                    # D6: E = exp(logits) bf16 (one op), zz sums (one op)
                    et = et_p.tile([128, 2, G * NQ], BF16, tag="et")
                    zz = zz_p.tile([128, 2, G], F32, tag="zz")
                    nc.scalar.activation(
                        out=et[:], in_=ps_l[:], func=AF.Exp)
                    nc.vector.tensor_reduce(
                        out=zz[:],
                        in_=et[:].rearrange("p t (g q) -> p t g q", g=G),
                        axis=AX.X,
                        op=OP.add,
                    )

                    # D8: u = hk.T @ E per 2 d-tiles; v = u * hq; vr = sum_q
                    vr_all = vr_p.tile([128, DT, G], F32, tag="vr")
                    for mp in range(4):
                        ps_u = up_p.tile([128, 512], F32, tag="up")
                        for mi in range(2):
                            m = mp * 2 + mi
                            for kt in range(2):
                                nc.tensor.matmul(
                                    ps_u[:, mi * 256 : (mi + 1) * 256],
                                    lhsT=hk[:, bi * 2 + kt, m * 128 : (m + 1) * 128],
                                    rhs=et[:, kt, :],
                                    start=(kt == 0),
                                    stop=(kt == 1),
                                )
                        v = v_p.tile([128, 2, G, NQ], BF16, tag="v")
                        nc.vector.tensor_tensor(
                            out=v[:],
                            in0=ps_u[:].rearrange("p (m g q) -> p m g q", m=2, g=G),
                            in1=hqT[
                                :, mp * 2 : mp * 2 + 2, None, b * NQ : (b + 1) * NQ
                            ].to_broadcast([128, 2, G, NQ]),
                            op=OP.mult,
                        )
                        nc.vector.tensor_reduce(
                            out=vr_all[:, mp * 2 : mp * 2 + 2, :], in_=v[:],
                            axis=AX.X, op=OP.add,
                        )

                    # D7 (late so the PE never waits on it): Z_g over
                    # k-partitions, then one fused pooled scale.
                    ps_z = mm_p.tile([128, 512], F32, tag="mm")
                    for kt in range(2):
                        nc.tensor.matmul(
                            ps_z[:1, :G],
                            lhsT=ones_sb[:],
                            rhs=zz[:, kt, :],
                            start=(kt == 0),
                            stop=(kt == 1),
                        )
                    zinv = zn_p.tile([1, G], F32, tag="zinv")
                    nc.vector.reciprocal(zinv[:1, :], ps_z[:1, :G])
                    zbro = zn_p.tile([128, G], F32, tag="zbro")
                    nc.gpsimd.partition_broadcast(zbro[:], zinv[:1, :], channels=128)
                    with nc.allow_low_precision(reason="bf16 pooled"):
                        nc.vector.tensor_tensor(
                            out=poT[:, :, :, b],
                            in0=vr_all[:],
                            in1=zbro[:, None, :].to_broadcast([128, DT, G]),
                            op=OP.mult,
                        )

                    if bi == 0:
                        if p < 3:
                            for t in range(2, 4):
                                xrow_tiles[("k", t)] = gather(
                                    idxk_sb, (p + 1) * 4 + t)
                        emit_weight_chunk(wstep); wstep += 1
                        emit_weight_chunk(wstep); wstep += 1

                if p < 3:
                    for t in range(4):
                        transpose_x(xrow_tiles.pop(("k", t)), xkT_next, t)
                emit_weight_chunk(wstep); wstep += 1
                if p < 3:
                    hk_cur, hkT_cur = project_k_pair(xkT_next)

            # attention pools (incl. all PSUM) are dead now
            actx.close()
            fctx = contextlib.ExitStack()
            fo_p = fctx.enter_context(tc.tile_pool(name="fo", bufs=2, space="PSUM"))

            # ---- phase F: out [8, 300] = pooled_flat @ Wout + bout ----
            ps_o = fo_p.tile([128, 512], F32, tag="fo")
            for g in range(G):
                for m in range(DT):
                    t = g * DT + m
                    nc.tensor.matmul(
                        ps_o[:BL, :N_OUT],
                        lhsT=poT[:, m, g, :],
                        rhs=wout_sb[:, t, :],
                        start=(t == 0),
                        stop=(t == NWOUT - 1),
                    )
            out_sb = consts.tile([BL, N_OUT], BF16, tag="out_sb")
            with nc.allow_low_precision(reason="bf16 out"):
                nc.vector.tensor_tensor(
                    out=out_sb[:], in0=ps_o[:BL, :N_OUT], in1=bout_sb[:], op=OP.add
                )

            # ---- phase G: sim + log_softmax (no max shift; sim is O(+-5)) --
            outT = consts.tile([128, 3, BL], BF16, tag="outT")
            for c, rows in enumerate(N_CHUNKS):
                psT = fo_p.tile([128, 128], BF16, tag="foT")
                nc.tensor.transpose(
                    psT[:rows, :BL],
                    out_sb[:, c * 128 : c * 128 + rows],
                    ident[:BL, :BL],
                )
                nc.scalar.activation(out=outT[:rows, c, :], in_=psT[:rows, :BL],
                                     func=AF.Copy)

            zs8 = consts.tile([BL, NA_CH], F32, tag="zs8")
            zs = consts.tile([BL, 1], F32, tag="zs")
            zsi = consts.tile([BL, 1], F32, tag="zsi")
            nlnz = consts.tile([BL, 1], F32, tag="nlnz")
            final_sb = consts.tile([BL, N_ANS], F32, tag="final")

            simp_tiles = []
            esc_p = ctx.enter_context(tc.tile_pool(name="esc", bufs=2))
            fctx.close()  # free F/outT PSUM banks before claiming all 8
            sim_p = ctx.enter_context(tc.tile_pool(name="simp", bufs=NA_CH,
                                                   space="PSUM"))
            for a in range(NA_CH):
                ps_s = sim_p.tile([128, NA_W], F32, tag="simp")
                for c, rows in enumerate(N_CHUNKS):
                    nc.tensor.matmul(
                        ps_s[:BL, :],
                        lhsT=outT[:rows, c, :],
                        rhs=glove_sb[:rows, c, a * NA_W : (a + 1) * NA_W],
                        start=(c == 0),
                        stop=(c == 2),
                    )
                esc = esc_p.tile([BL, NA_W], BF16, tag="esc")
                nc.scalar.activation(out=esc[:], in_=ps_s[:BL, :], func=AF.Exp)
                nc.vector.tensor_reduce(
                    out=zs8[:, a : a + 1], in_=esc[:], axis=AX.X, op=OP.add
                )
                simp_tiles.append(ps_s)

            nc.vector.tensor_reduce(out=zs[:], in_=zs8[:], axis=AX.X, op=OP.add)
            nc.vector.reciprocal(zsi[:], zs[:])
            nc.scalar.activation(out=nlnz[:], in_=zsi[:], func=AF.Ln)
            # final = sim - lnZ, 4 chunks on DVE + 4 on Act, then 2 DMAs
            for a in range(NA_CH):
                span = slice(a * NA_W, (a + 1) * NA_W)
                if a % 2 == 0:
                    nc.vector.tensor_scalar(
                        out=final_sb[:, span], in0=simp_tiles[a][:BL, :],
                        scalar1=nlnz[:], scalar2=None,
                        op0=OP.add,
                    )
                else:
                    nc.scalar.activation(
                        out=final_sb[:, span], in_=simp_tiles[a][:BL, :],
                        func=AF.Identity, bias=nlnz[:],
                    )
                if a == 3:
                    nc.sync.dma_start(out_d[:, : 4 * NA_W], final_sb[:, : 4 * NA_W])
            nc.sync.dma_start(out_d[:, 4 * NA_W :], final_sb[:, 4 * NA_W :])

    nc.compile()
    return nc


_NC = None


def _get_nc():
    global _NC
    if _NC is None:
        _NC = build_kernel()
    return _NC


def make_in_maps(inputs):
    import ml_dtypes

    bf = ml_dtypes.bfloat16
    he_q = np.asarray(inputs["he_ques"]).astype(np.int32)   # [64, 32]
    he_k = np.asarray(inputs["he_kg"]).astype(np.int32)     # [64, 256]
    emb0 = np.asarray(inputs["emb"], dtype=np.float32)
    emb = np.zeros((VOCAB, EA), dtype=bf)
    emb[:, :E] = emb0.astype(bf)
    emb[:, E] = np.ones((), dtype=bf)                       # bias column
    wq = np.zeros((EA, H), dtype=bf)
    wq[:E] = np.asarray(inputs["Wq"], np.float32).astype(bf)
    wq[E] = np.asarray(inputs["bq"], np.float32).astype(bf)
    wk = np.zeros((EA, H), dtype=bf)
    wk[:E] = np.asarray(inputs["Wk"], np.float32).astype(bf)
    wk[E] = np.asarray(inputs["bk"], np.float32).astype(bf)
    watt = np.ascontiguousarray(
        np.asarray(inputs["Watt"], np.float32).reshape(DT, 128, G)
        .transpose(1, 0, 2)).astype(bf)                     # [128, DT, G]
    wout = np.ascontiguousarray(
        np.asarray(inputs["Wout"], np.float32)).astype(bf)
    bout = np.ascontiguousarray(
        np.broadcast_to(np.asarray(inputs["bout"], np.float32), (BL, N_OUT)))
    glovet = np.ascontiguousarray(
        np.asarray(inputs["glove_cands"], np.float32).T).astype(bf)  # [300,4000]

    in_maps = []
    for i in range(N_CORES):
        iq = he_q[i * BL : (i + 1) * BL].reshape(-1)        # [256]
        ik = he_k[i * BL : (i + 1) * BL].reshape(-1)        # [2048]
        in_maps.append({
            "emb": emb,
            "idx_q": np.ascontiguousarray(iq.reshape(TQ_TILES, 128).T),
            "idx_k": np.ascontiguousarray(ik.reshape(TK_TILES, 128).T),
            "wq": wq,
            "wk": wk,
            "watt": watt,
            "wout": wout,
            "bout": bout,
            "glovet": glovet,
        })
    return in_maps


def kernel(**inputs) -> np.ndarray:
    nc = _get_nc()
    in_maps = make_in_maps(inputs)
    res = run_bass_kernel_spmd(nc, in_maps, list(range(N_CORES)))
    return np.concatenate(
        [np.asarray(res.results[i]["out"], np.float32) for i in range(N_CORES)],
        axis=0,
    )
